# revision 7
# baseline (speedup 1.0000x reference)
"""Self-contained Trainium2 kernel for the GroupNorm+Attention block.

Reference computation (B=2, H=W=64, C=512, GROUPS=32):
    hn = group_norm(x)            # per (batch, group) stats over (H, W, C/G)
    q, k, v = hn@wq+bq, hn@wk+bk, hn@wv+bv
    s = q @ k^T / sqrt(C)         # per batch, N=4096 tokens
    p = softmax(s)
    out = x + (p @ v) @ wp + bp

Sharding: 8 cores = 2 batches x 4 row-blocks of 1024 query rows.
Each core redundantly computes its batch's GN stats, K^T and V (cheap
vs collectives), and its own 1024-row slice of Q / attention / output.

Key design points:
 - Everything is built on the transposed layout x^T [C, N] so that every
   GEMM's contraction dim lands on partitions with zero device transposes:
     Q^T = wq'^T_fold @ x^T,  K^T likewise,  V = x^T_chunks.T @ wv'
     S^T[j,i] = K^T.T @ Q^T   (softmax denominator via ones-vector matmul)
     O^T[c,i] = V.T-chunks @ P~^T,  Y^T = wp.T-chunks @ O^T
 - GroupNorm is folded into the QKV weights: xn = x*A + Bv per channel,
   so w' = A*w (row scale) and bias' = Bv@w + b. 1/sqrt(C) folds into Q.
 - exp() without max subtraction (scores are O(1) here; fp32 exp is safe).
 - Matmuls run in bf16 (f32 PSUM accumulation); stats, softmax denominator,
   residual and output stay f32. Final output error ~1e-4 (residual "x"
   dominates the output, attention path is small).
"""

import sys

sys.path.insert(0, "/opt/trn_rl_repo")

import numpy as np

B, Hh, Ww, C = 2, 64, 64, 512
N = Hh * Ww  # 4096 tokens per batch
G, CPG = 32, 16
EPS = 1e-5
P = 128
CH = C // P  # 4 channel chunks
NJ = N // P  # 32 token chunks
FT = 512  # matmul free-dim tile
NS = N // FT  # 8
NQ = N // 4  # 1024 query rows per core
QS = NQ // FT  # 2
INV_SQRT_C = 1.0 / float(np.sqrt(C))

_CACHE = {}


def _build():
    import concourse.bass as bass  # noqa: F401
    import concourse.tile as tile
    from concourse import bacc, mybir

    fp = mybir.dt.float32
    bf = mybir.dt.bfloat16
    AF = mybir.ActivationFunctionType
    ALU = mybir.AluOpType

    nc = bacc.Bacc(None, target_bir_lowering=False, debug=False)

    xT_ext = nc.declare_dram_parameter("xT", [C, N], fp, isOutput=False)
    xq_ext = nc.declare_dram_parameter("xq", [C, NQ], fp, isOutput=False)
    w_ext = {k: nc.declare_dram_parameter(f"w{k}", [C, C], fp, isOutput=False) for k in "qkvp"}
    b_ext = {k: nc.declare_dram_parameter(f"b{k}", [C, 1], fp, isOutput=False) for k in "qkvp"}
    gam_ext = nc.declare_dram_parameter("gamma", [C, 1], fp, isOutput=False)
    bet_ext = nc.declare_dram_parameter("beta", [C, 1], fp, isOutput=False)
    fmat_ext = nc.declare_dram_parameter("fmat", [C, G], fp, isOutput=False)
    emat_ext = nc.declare_dram_parameter("emat", [G, C], fp, isOutput=False)
    ones_ext = nc.declare_dram_parameter("ones", [P, P], fp, isOutput=False)
    out_ext = nc.declare_dram_parameter("out", [C, NQ], fp, isOutput=True)

    with tile.TileContext(nc) as tc:
        with (
            tc.tile_pool(name="persist", bufs=1) as sb,
            tc.tile_pool(name="stream", bufs=2) as st,
            tc.tile_pool(name="psb", bufs=4, space="PSUM") as psb,
            tc.tile_pool(name="pss", bufs=2, space="PSUM") as pss,
        ):
            # ---------- constants / vectors ----------
            ones_f = sb.tile([P, P], fp, tag="ones_f")
            nc.sync.dma_start(out=ones_f, in_=ones_ext[:, :])
            ones_b = sb.tile([P, P], bf, tag="ones_b")
            nc.gpsimd.tensor_copy(out=ones_b, in_=ones_f)
            emat_sb = sb.tile([G, C], fp, tag="emat_sb")
            nc.sync.dma_start(out=emat_sb, in_=emat_ext[:, :])

            fmat_sb, gam, bet, bcol = [], [], [], {k: [] for k in "qkvp"}
            for ci in range(CH):
                cs = slice(ci * P, (ci + 1) * P)
                t = sb.tile([P, G], fp, tag=f"fmat{ci}", name=f"fmat{ci}")
                nc.sync.dma_start(out=t, in_=fmat_ext[cs, :])
                fmat_sb.append(t)
                t = sb.tile([P, 1], fp, tag=f"gam{ci}", name=f"gam{ci}")
                nc.sync.dma_start(out=t, in_=gam_ext[cs, :])
                gam.append(t)
                t = sb.tile([P, 1], fp, tag=f"bet{ci}", name=f"bet{ci}")
                nc.sync.dma_start(out=t, in_=bet_ext[cs, :])
                bet.append(t)
                for k in "qkvp":
                    t = sb.tile([P, 1], fp, tag=f"b{k}{ci}", name=f"b{k}{ci}")
                    nc.sync.dma_start(out=t, in_=b_ext[k][cs, :])
                    bcol[k].append(t)

            # ---------- load x^T, GN stats (bn_stats), cast to bf16 ----------
            xtbf = [sb.tile([P, N], bf, tag=f"xtbf{ci}", name=f"xtbf{ci}") for ci in range(CH)]
            srhs = []  # [P, 3] per chunk: (mean, var, mean^2) per channel
            for ci in range(CH):
                st6 = sb.tile([P, 8, 6], fp, tag=f"st6_{ci}", name=f"st6_{ci}")
                for nsub in range(4):
                    xf = st.tile([P, 1024], fp, tag="xt_f32", name=f"xtf_{ci}_{nsub}")
                    nc.sync.dma_start(out=xf, in_=xT_ext[ci * P:(ci + 1) * P, nsub * 1024:(nsub + 1) * 1024])
                    for s2 in range(2):
                        nc.vector.bn_stats(
                            out=st6[:, nsub * 2 + s2, :],
                            in_=xf[:, s2 * 512:(s2 + 1) * 512],
                        )
                    nc.gpsimd.tensor_copy(
                        out=xtbf[ci][:, nsub * 1024:(nsub + 1) * 1024], in_=xf
                    )
                mv = sb.tile([P, 2], fp, tag=f"mv{ci}", name=f"mv{ci}")
                nc.vector.bn_aggr(out=mv, in_=st6)
                sr = sb.tile([P, 3], fp, tag=f"sr{ci}", name=f"sr{ci}")
                nc.vector.tensor_copy(out=sr[:, 0:2], in_=mv)
                nc.vector.tensor_mul(sr[:, 2:3], mv[:, 0:1], mv[:, 0:1])
                srhs.append(sr)

            # ---------- group stats: [32] mu_g, E[var]_g, E[mu^2]_g ----------
            ps_g = pss.tile([G, 3], fp, tag="small", name="ps_g")
            for ci in range(CH):
                nc.tensor.matmul(ps_g, fmat_sb[ci], srhs[ci], start=(ci == 0), stop=(ci == CH - 1))
            sg = sb.tile([G, 3], fp, tag="sg")
            nc.vector.tensor_copy(out=sg, in_=ps_g)
            varg = sb.tile([G, 1], fp, tag="varg")
            nc.vector.tensor_add(varg, sg[:, 1:2], sg[:, 2:3])  # E[var] + E[mu^2]
            musq = sb.tile([G, 1], fp, tag="musq")
            nc.vector.tensor_mul(musq, sg[:, 0:1], sg[:, 0:1])
            nc.vector.tensor_sub(varg, varg, musq)
            grhs = sb.tile([G, 2], fp, tag="grhs")  # (rsd_g, mu_g)
            eps_t = sb.tile([G, 1], fp, tag="eps_t")
            nc.vector.memset(eps_t, EPS)
            nc.scalar.activation(out=grhs[:, 0:1], in_=varg, func=AF.Sqrt, bias=eps_t, scale=1.0)
            nc.vector.reciprocal(out=grhs[:, 0:1], in_=grhs[:, 0:1])
            nc.vector.tensor_copy(out=grhs[:, 1:2], in_=sg[:, 0:1])

            # ---------- broadcast to channels; A, Aq, Bv columns ----------
            Acol, Aqcol, Bvcol = [], [], []
            for ci in range(CH):
                ps_bc = pss.tile([P, 2], fp, tag="small", name=f"ps_bc{ci}")
                nc.tensor.matmul(ps_bc, emat_sb[:, ci * P:(ci + 1) * P], grhs, start=True, stop=True)
                a = sb.tile([P, 1], fp, tag=f"A{ci}", name=f"A{ci}")
                nc.vector.tensor_mul(a, ps_bc[:, 0:1], gam[ci])
                aq = sb.tile([P, 1], fp, tag=f"Aq{ci}", name=f"Aq{ci}")
                nc.vector.tensor_scalar_mul(out=aq, in0=a, scalar1=INV_SQRT_C)
                bv_ = sb.tile([P, 1], fp, tag=f"Bv{ci}", name=f"Bv{ci}")
                nc.vector.tensor_mul(bv_, ps_bc[:, 1:2], a)
                nc.vector.tensor_sub(bv_, bet[ci], bv_)
                Acol.append(a)
                Aqcol.append(aq)
                Bvcol.append(bv_)

            # ---------- weights: bias folds + row-scaled bf16 casts ----------
            wbf = {k: [] for k in "qkvp"}
            biasq, biask, bvpcol, biasp = [], [], [], []
            for k in ("q", "k", "v", "p"):
                wf_chunks = []
                for ci in range(CH):
                    wf = st.tile([P, C], fp, tag="w_f32", name=f"wf_{k}{ci}", bufs=4)
                    nc.sync.dma_start(out=wf, in_=w_ext[k][ci * P:(ci + 1) * P, :])
                    wf_chunks.append(wf)
                    wb = sb.tile([P, C], bf, tag=f"w{k}b{ci}", name=f"w{k}b{ci}")
                    scale_col = Aqcol[ci] if k == "q" else Acol[ci]
                    if k == "p":
                        nc.vector.tensor_copy(out=wb, in_=wf)
                    else:
                        nc.vector.tensor_scalar_mul(out=wb, in0=wf, scalar1=scale_col)
                    wbf[k].append(wb)
                for co in range(CH):
                    ps_b = pss.tile([P, 1], fp, tag="small", name=f"ps_b{k}{co}")
                    for ci in range(CH):
                        rhs_vec = Bvcol[ci] if k != "p" else bvpcol[ci]
                        nc.tensor.matmul(
                            ps_b,
                            wf_chunks[ci][:, co * P:(co + 1) * P],
                            rhs_vec,
                            start=(ci == 0),
                            stop=(ci == CH - 1),
                        )
                    bc_ = sb.tile([P, 1], fp, tag=f"bias{k}{co}", name=f"bias{k}{co}")
                    if k == "q":
                        nc.vector.tensor_scalar(
                            out=bc_, in0=ps_b, scalar1=bcol["q"][co],
                            scalar2=INV_SQRT_C, op0=ALU.add, op1=ALU.mult,
                        )
                        biasq.append(bc_)
                    elif k == "k":
                        nc.vector.tensor_add(bc_, ps_b, bcol["k"][co])
                        biask.append(bc_)
                    elif k == "v":
                        nc.vector.tensor_add(bc_, ps_b, bcol["v"][co])
                        bvpcol.append(bc_)
                    else:
                        nc.vector.tensor_add(bc_, ps_b, bcol["p"][co])
                        biasp.append(bc_)

            # ---------- xq load + cast ----------
            xqbf = []
            for ci in range(CH):
                xqf = st.tile([P, NQ], fp, tag="xq_f32", name=f"xqf{ci}", bufs=1)
                nc.sync.dma_start(out=xqf, in_=xq_ext[ci * P:(ci + 1) * P, :])
                t = sb.tile([P, NQ], bf, tag=f"xqbf{ci}", name=f"xqbf{ci}")
                nc.gpsimd.tensor_copy(out=t, in_=xqf)
                xqbf.append(t)

            # ---------- Q^T [C, NQ] ----------
            qtbf = [sb.tile([P, NQ], bf, tag=f"qt{co}", name=f"qt{co}") for co in range(CH)]
            for co in range(CH):
                for s in range(QS):
                    ps = psb.tile([P, FT], fp, tag="big", name=f"ps_q{co}_{s}")
                    for ci in range(CH):
                        nc.tensor.matmul(
                            ps, wbf["q"][ci][:, co * P:(co + 1) * P],
                            xqbf[ci][:, s * FT:(s + 1) * FT],
                            start=(ci == 0), stop=(ci == CH - 1),
                        )
                    nc.vector.tensor_scalar(
                        out=qtbf[co][:, s * FT:(s + 1) * FT], in0=ps,
                        scalar1=biasq[co], scalar2=None, op0=ALU.add,
                    )

            # ---------- K^T [C, N] ----------
            ktbf = [sb.tile([P, N], bf, tag=f"kt{co}", name=f"kt{co}") for co in range(CH)]
            for co in range(CH):
                for s in range(NS):
                    ps = psb.tile([P, FT], fp, tag="big", name=f"ps_k{co}_{s}")
                    for ci in range(CH):
                        nc.tensor.matmul(
                            ps, wbf["k"][ci][:, co * P:(co + 1) * P],
                            xtbf[ci][:, s * FT:(s + 1) * FT],
                            start=(ci == 0), stop=(ci == CH - 1),
                        )
                    nc.vector.tensor_scalar(
                        out=ktbf[co][:, s * FT:(s + 1) * FT], in0=ps,
                        scalar1=biask[co], scalar2=None, op0=ALU.add,
                    )

            # ---------- V [N, C] (no bias; folded into proj bias) ----------
            vbf = [sb.tile([P, C], bf, tag=f"v{nj}", name=f"v{nj}") for nj in range(NJ)]
            for nj in range(NJ):
                ps = psb.tile([P, FT], fp, tag="big", name=f"ps_v{nj}")
                for ci in range(CH):
                    nc.tensor.matmul(
                        ps, xtbf[ci][:, nj * P:(nj + 1) * P], wbf["v"][ci],
                        start=(ci == 0), stop=(ci == CH - 1),
                    )
                if nj % 2 == 0:
                    nc.scalar.activation(out=vbf[nj], in_=ps, func=AF.Copy)
                else:
                    nc.vector.tensor_copy(out=vbf[nj], in_=ps)

            # ---------- attention + projection, per 512-query block ----------
            for ib in range(QS):
                isl = slice(ib * FT, (ib + 1) * FT)
                # S^T tiles -> exp -> P~^T (bf16)
                pt = [
                    st.tile([P, FT], bf, tag=f"pt{j}", name=f"pt{ib}_{j}", bufs=1)
                    for j in range(NJ)
                ]
                for j in range(NJ):
                    ps = psb.tile([P, FT], fp, tag="big", name=f"ps_s{ib}_{j}")
                    for c in range(CH):
                        nc.tensor.matmul(
                            ps, ktbf[c][:, j * P:(j + 1) * P], qtbf[c][:, isl],
                            start=(c == 0), stop=(c == CH - 1),
                        )
                    nc.scalar.activation(out=pt[j], in_=ps, func=AF.Exp)
                # softmax denominator: ones^T @ P~^T, then reciprocal+broadcast
                ps_d = pss.tile([1, FT], fp, tag="denom", name=f"ps_d{ib}")
                for j in range(NJ):
                    nc.tensor.matmul(ps_d, ones_b[:, 0:1], pt[j], start=(j == 0), stop=(j == NJ - 1))
                rd_row = st.tile([1, FT], fp, tag="rd_row", name=f"rd_row{ib}")
                nc.vector.reciprocal(out=rd_row, in_=ps_d)
                ps_bc = psb.tile([P, FT], fp, tag="big", name=f"ps_rbc{ib}")
                nc.tensor.matmul(ps_bc, ones_f[0:1, :], rd_row, start=True, stop=True)
                rd_bc = st.tile([P, FT], fp, tag="rd_bc", name=f"rd_bc{ib}")
                nc.vector.tensor_copy(out=rd_bc, in_=ps_bc)
                # O^T[c, i] = sum_j V[j,c-chunk]^T P~^T[j, i], then /denom
                otbf = []
                for c in range(CH):
                    ps = psb.tile([P, FT], fp, tag="big", name=f"ps_o{ib}_{c}")
                    for j in range(NJ):
                        nc.tensor.matmul(
                            ps, vbf[j][:, c * P:(c + 1) * P], pt[j],
                            start=(j == 0), stop=(j == NJ - 1),
                        )
                    ot = st.tile([P, FT], bf, tag=f"ot{c}", name=f"ot{ib}_{c}", bufs=1)
                    nc.vector.tensor_mul(ot, ps, rd_bc)
                    otbf.append(ot)
                # Y^T[co, i] = wp^T-chunks @ O^T + bias' + residual
                for co in range(CH):
                    ps = psb.tile([P, FT], fp, tag="big", name=f"ps_y{ib}_{co}")
                    for c in range(CH):
                        nc.tensor.matmul(
                            ps, wbf["p"][c][:, co * P:(co + 1) * P], otbf[c],
                            start=(c == 0), stop=(c == CH - 1),
                        )
                    res = st.tile([P, FT], fp, tag="res", name=f"res{ib}_{co}", bufs=1)
                    nc.sync.dma_start(out=res, in_=xq_ext[co * P:(co + 1) * P, isl])
                    yt = st.tile([P, FT], fp, tag="yt", name=f"yt{ib}_{co}")
                    nc.vector.tensor_scalar(
                        out=yt, in0=ps, scalar1=biasp[co], scalar2=None, op0=ALU.add
                    )
                    nc.vector.tensor_add(yt, yt, res)
                    nc.sync.dma_start(out=out_ext[co * P:(co + 1) * P, isl], in_=yt)

    nc.finalize()
    return nc


def _get_nc():
    if "nc" not in _CACHE:
        _CACHE["nc"] = _build()
    return _CACHE["nc"]


def kernel(x, gamma, beta, wq, bq, wk, bk, wv, bv, wp, bp):
    from concourse.bass_utils import run_bass_kernel_spmd

    nc = _get_nc()

    x = np.asarray(x, dtype=np.float32)
    fmat = np.zeros((C, G), np.float32)
    emat = np.zeros((G, C), np.float32)
    for c in range(C):
        fmat[c, c // CPG] = 1.0 / CPG
        emat[c // CPG, c] = 1.0
    ones = np.ones((P, P), np.float32)

    def colv(v):
        return np.ascontiguousarray(np.asarray(v, np.float32).reshape(C, 1))

    common = {
        "wq": np.asarray(wq, np.float32), "wk": np.asarray(wk, np.float32),
        "wv": np.asarray(wv, np.float32), "wp": np.asarray(wp, np.float32),
        "bq": colv(bq), "bk": colv(bk), "bv": colv(bv), "bp": colv(bp),
        "gamma": colv(gamma), "beta": colv(beta),
        "fmat": fmat, "emat": emat, "ones": ones,
    }

    xT = [np.ascontiguousarray(x[b].reshape(N, C).T) for b in range(B)]
    in_maps = []
    for core in range(8):
        b, r = core // 4, core % 4
        m = dict(common)
        m["xT"] = xT[b]
        m["xq"] = np.ascontiguousarray(xT[b][:, r * NQ:(r + 1) * NQ])
        in_maps.append(m)

    res = run_bass_kernel_spmd(nc, in_maps, core_ids=list(range(8)))

    out = np.empty((B, N, C), np.float32)
    for core in range(8):
        b, r = core // 4, core % 4
        out[b, r * NQ:(r + 1) * NQ, :] = res.results[core]["out"].T
    return out.reshape(B, Hh, Ww, C)


# revision 12
# speedup vs baseline: 1.0843x; 1.0843x over previous
"""Self-contained Trainium2 kernel for the GroupNorm+Attention block.

Reference computation (B=2, H=W=64, C=512, GROUPS=32):
    hn = group_norm(x)            # per (batch, group) stats over (H, W, C/G)
    q, k, v = hn@wq+bq, hn@wk+bk, hn@wv+bv
    s = q @ k^T / sqrt(C)         # per batch, N=4096 tokens
    p = softmax(s)
    out = x + (p @ v) @ wp + bp

Sharding: 8 cores = 2 batches x 4 row-blocks of 1024 query rows.
Each core redundantly computes its batch's GN stats, K^T and V (cheap
vs collectives), and its own 1024-row slice of Q / attention / output.

Key design points:
 - Everything is built on the transposed layout x^T [C, N] so that every
   GEMM's contraction dim lands on partitions with zero device transposes:
     Q^T = wq'^T_fold @ x^T,  K^T likewise,  V = x^T_chunks.T @ wv'
     S^T[j,i] = K^T.T @ Q^T   (softmax denominator via ones-vector matmul)
     O^T[c,i] = V.T-chunks @ P~^T,  Y^T = wp.T-chunks @ O^T
 - GroupNorm is folded into the QKV weights: xn = x*A + Bv per channel,
   so w' = A*w (row scale) and bias' = Bv@w + b. 1/sqrt(C) folds into Q.
 - exp() without max subtraction (scores are O(1) here; fp32 exp is safe).
 - Matmuls run in bf16 (f32 PSUM accumulation); stats, softmax denominator,
   residual and output stay f32. Final output error ~1e-4 (residual "x"
   dominates the output, attention path is small).
"""

import sys

sys.path.insert(0, "/opt/trn_rl_repo")

import numpy as np

B, Hh, Ww, C = 2, 64, 64, 512
N = Hh * Ww  # 4096 tokens per batch
G, CPG = 32, 16
EPS = 1e-5
P = 128
CH = C // P  # 4 channel chunks
NJ = N // P  # 32 token chunks
FT = 512  # matmul free-dim tile
NS = N // FT  # 8
NQ = N // 4  # 1024 query rows per core
QS = NQ // FT  # 2
INV_SQRT_C = 1.0 / float(np.sqrt(C))

_CACHE = {}


def _build():
    import concourse.bass as bass  # noqa: F401
    import concourse.tile as tile
    from concourse import bacc, mybir

    fp = mybir.dt.float32
    bf = mybir.dt.bfloat16
    AF = mybir.ActivationFunctionType
    ALU = mybir.AluOpType

    nc = bacc.Bacc(None, target_bir_lowering=False, debug=False)

    xT_ext = nc.declare_dram_parameter("xT", [C, N], fp, isOutput=False)
    xq_ext = nc.declare_dram_parameter("xq", [C, NQ], fp, isOutput=False)
    w_ext = {k: nc.declare_dram_parameter(f"w{k}", [C, C], fp, isOutput=False) for k in "qkvp"}
    vecs_ext = nc.declare_dram_parameter("vecs", [C, 6], fp, isOutput=False)
    fmat_ext = nc.declare_dram_parameter("fmat", [C, G], fp, isOutput=False)
    emat_ext = nc.declare_dram_parameter("emat", [G, C], fp, isOutput=False)
    ones_ext = nc.declare_dram_parameter("ones", [P, P], fp, isOutput=False)
    out_ext = nc.declare_dram_parameter("out", [C, NQ], fp, isOutput=True)

    with tile.TileContext(nc) as tc:
        with (
            tc.tile_pool(name="persist", bufs=1) as sb,
            tc.tile_pool(name="stream", bufs=2) as st,
            tc.tile_pool(name="psb", bufs=4, space="PSUM") as psb,
            tc.tile_pool(name="pss", bufs=2, space="PSUM") as pss,
        ):
            # ---------- load x^T (first: bandwidth-critical), GN stats ----------
            xtbf = [sb.tile([P, N], bf, tag=f"xtbf{ci}", name=f"xtbf{ci}") for ci in range(CH)]
            srhs = []  # [P, 3] per chunk: (mean, var, mean^2) per channel
            for ci in range(CH):
                st6 = sb.tile([P, 8, 6], fp, tag=f"st6_{ci}", name=f"st6_{ci}")
                for nsub in range(4):
                    xf = st.tile([P, 1024], fp, tag="xt_f32", name=f"xtf_{ci}_{nsub}", bufs=3)
                    dma_eng = nc.sync if nsub % 2 == 0 else nc.gpsimd
                    dma_eng.dma_start(out=xf, in_=xT_ext[ci * P:(ci + 1) * P, nsub * 1024:(nsub + 1) * 1024])
                    for s2 in range(2):
                        nc.vector.bn_stats(
                            out=st6[:, nsub * 2 + s2, :],
                            in_=xf[:, s2 * 512:(s2 + 1) * 512],
                        )
                    nc.gpsimd.tensor_copy(
                        out=xtbf[ci][:, nsub * 1024:(nsub + 1) * 1024], in_=xf
                    )
                mv = sb.tile([P, 2], fp, tag=f"mv{ci}", name=f"mv{ci}")
                nc.vector.bn_aggr(out=mv, in_=st6)
                sr = sb.tile([P, 3], fp, tag=f"sr{ci}", name=f"sr{ci}")
                nc.vector.tensor_copy(out=sr[:, 0:2], in_=mv)
                nc.vector.tensor_mul(sr[:, 2:3], mv[:, 0:1], mv[:, 0:1])
                srhs.append(sr)

            # ---------- constants / vectors (after xT streaming: tiny DMAs
            # must not head-of-line-block the bandwidth-critical x^T loads) ----------
            ones_f = sb.tile([P, P], fp, tag="ones_f")
            nc.sync.dma_start(out=ones_f, in_=ones_ext[:, :])
            ones_b = sb.tile([P, P], bf, tag="ones_b")
            nc.gpsimd.tensor_copy(out=ones_b, in_=ones_f)
            emat_sb = sb.tile([G, C], fp, tag="emat_sb")
            nc.sync.dma_start(out=emat_sb, in_=emat_ext[:, :])

            fmat_sb, gam, bet, bcol = [], [], [], {k: [] for k in "qkvp"}
            for ci in range(CH):
                cs = slice(ci * P, (ci + 1) * P)
                t = sb.tile([P, G], fp, tag=f"fmat{ci}", name=f"fmat{ci}")
                nc.sync.dma_start(out=t, in_=fmat_ext[cs, :])
                fmat_sb.append(t)
                v6 = sb.tile([P, 6], fp, tag=f"vecs{ci}", name=f"vecs{ci}")
                nc.sync.dma_start(out=v6, in_=vecs_ext[cs, :])
                gam.append(v6[:, 0:1])
                bet.append(v6[:, 1:2])
                for j, k in enumerate("qkvp"):
                    bcol[k].append(v6[:, 2 + j:3 + j])

            # ---------- group stats: [32] mu_g, E[var]_g, E[mu^2]_g ----------
            ps_g = pss.tile([G, 3], fp, tag="small", name="ps_g")
            for ci in range(CH):
                nc.tensor.matmul(ps_g, fmat_sb[ci], srhs[ci], start=(ci == 0), stop=(ci == CH - 1))
            sg = sb.tile([G, 3], fp, tag="sg")
            nc.vector.tensor_copy(out=sg, in_=ps_g)
            varg = sb.tile([G, 1], fp, tag="varg")
            nc.vector.tensor_add(varg, sg[:, 1:2], sg[:, 2:3])  # E[var] + E[mu^2]
            musq = sb.tile([G, 1], fp, tag="musq")
            nc.vector.tensor_mul(musq, sg[:, 0:1], sg[:, 0:1])
            nc.vector.tensor_sub(varg, varg, musq)
            grhs = sb.tile([G, 2], fp, tag="grhs")  # (rsd_g, mu_g)
            eps_t = sb.tile([G, 1], fp, tag="eps_t")
            nc.vector.memset(eps_t, EPS)
            nc.scalar.activation(out=grhs[:, 0:1], in_=varg, func=AF.Sqrt, bias=eps_t, scale=1.0)
            nc.vector.reciprocal(out=grhs[:, 0:1], in_=grhs[:, 0:1])
            nc.vector.tensor_copy(out=grhs[:, 1:2], in_=sg[:, 0:1])

            # ---------- broadcast to channels; A, Aq, Bv columns ----------
            Acol, Aqcol, Bvcol = [], [], []
            for ci in range(CH):
                ps_bc = pss.tile([P, 2], fp, tag="small", name=f"ps_bc{ci}")
                nc.tensor.matmul(ps_bc, emat_sb[:, ci * P:(ci + 1) * P], grhs, start=True, stop=True)
                a = sb.tile([P, 1], fp, tag=f"A{ci}", name=f"A{ci}")
                nc.vector.tensor_mul(a, ps_bc[:, 0:1], gam[ci])
                aq = sb.tile([P, 1], fp, tag=f"Aq{ci}", name=f"Aq{ci}")
                nc.vector.tensor_scalar_mul(out=aq, in0=a, scalar1=INV_SQRT_C)
                bv_ = sb.tile([P, 1], fp, tag=f"Bv{ci}", name=f"Bv{ci}")
                nc.vector.tensor_mul(bv_, ps_bc[:, 1:2], a)
                nc.vector.tensor_sub(bv_, bet[ci], bv_)
                Acol.append(a)
                Aqcol.append(aq)
                Bvcol.append(bv_)

            # ---------- weights: bias folds + row-scaled bf16 casts ----------
            wbf = {k: [] for k in "qkvp"}
            biasq, biask, bvpcol, biasp = [], [], [], []
            for k in ("q", "k", "v", "p"):
                wf_chunks = []
                for ci in range(CH):
                    wf = st.tile([P, C], fp, tag="w_f32", name=f"wf_{k}{ci}", bufs=4)
                    nc.sync.dma_start(out=wf, in_=w_ext[k][ci * P:(ci + 1) * P, :])
                    wf_chunks.append(wf)
                    wb = sb.tile([P, C], bf, tag=f"w{k}b{ci}", name=f"w{k}b{ci}")
                    scale_col = Aqcol[ci] if k == "q" else Acol[ci]
                    if k == "p":
                        nc.vector.tensor_copy(out=wb, in_=wf)
                    else:
                        nc.vector.tensor_scalar_mul(out=wb, in0=wf, scalar1=scale_col)
                    wbf[k].append(wb)
                for co in range(CH):
                    ps_b = pss.tile([P, 1], fp, tag="small", name=f"ps_b{k}{co}")
                    for ci in range(CH):
                        rhs_vec = Bvcol[ci] if k != "p" else bvpcol[ci]
                        nc.tensor.matmul(
                            ps_b,
                            wf_chunks[ci][:, co * P:(co + 1) * P],
                            rhs_vec,
                            start=(ci == 0),
                            stop=(ci == CH - 1),
                        )
                    bc_ = sb.tile([P, 1], fp, tag=f"bias{k}{co}", name=f"bias{k}{co}")
                    if k == "q":
                        nc.vector.tensor_scalar(
                            out=bc_, in0=ps_b, scalar1=bcol["q"][co],
                            scalar2=INV_SQRT_C, op0=ALU.add, op1=ALU.mult,
                        )
                        biasq.append(bc_)
                    elif k == "k":
                        nc.vector.tensor_add(bc_, ps_b, bcol["k"][co])
                        biask.append(bc_)
                    elif k == "v":
                        nc.vector.tensor_add(bc_, ps_b, bcol["v"][co])
                        bvpcol.append(bc_)
                    else:
                        nc.vector.tensor_add(bc_, ps_b, bcol["p"][co])
                        biasp.append(bc_)

            # ---------- xq load + cast ----------
            xqbf = []
            for ci in range(CH):
                xqf = st.tile([P, NQ], fp, tag="xq_f32", name=f"xqf{ci}", bufs=1)
                nc.sync.dma_start(out=xqf, in_=xq_ext[ci * P:(ci + 1) * P, :])
                t = sb.tile([P, NQ], bf, tag=f"xqbf{ci}", name=f"xqbf{ci}")
                nc.gpsimd.tensor_copy(out=t, in_=xqf)
                xqbf.append(t)

            # ---------- Q^T [C, NQ] ----------
            qtbf = [sb.tile([P, NQ], bf, tag=f"qt{co}", name=f"qt{co}") for co in range(CH)]
            for co in range(CH):
                for s in range(QS):
                    ps = psb.tile([P, FT], fp, tag="big", name=f"ps_q{co}_{s}")
                    for ci in range(CH):
                        nc.tensor.matmul(
                            ps, wbf["q"][ci][:, co * P:(co + 1) * P],
                            xqbf[ci][:, s * FT:(s + 1) * FT],
                            start=(ci == 0), stop=(ci == CH - 1),
                        )
                    nc.vector.tensor_scalar(
                        out=qtbf[co][:, s * FT:(s + 1) * FT], in0=ps,
                        scalar1=biasq[co], scalar2=None, op0=ALU.add,
                    )

            # ---------- K^T [C, N] ----------
            ktbf = [sb.tile([P, N], bf, tag=f"kt{co}", name=f"kt{co}") for co in range(CH)]
            for co in range(CH):
                for s in range(NS):
                    ps = psb.tile([P, FT], fp, tag="big", name=f"ps_k{co}_{s}")
                    for ci in range(CH):
                        nc.tensor.matmul(
                            ps, wbf["k"][ci][:, co * P:(co + 1) * P],
                            xtbf[ci][:, s * FT:(s + 1) * FT],
                            start=(ci == 0), stop=(ci == CH - 1),
                        )
                    nc.vector.tensor_scalar(
                        out=ktbf[co][:, s * FT:(s + 1) * FT], in0=ps,
                        scalar1=biask[co], scalar2=None, op0=ALU.add,
                    )

            # ---------- V [N, C] (no bias; folded into proj bias) ----------
            vbf = [sb.tile([P, C], bf, tag=f"v{nj}", name=f"v{nj}") for nj in range(NJ)]
            for nj in range(NJ):
                ps = psb.tile([P, FT], fp, tag="big", name=f"ps_v{nj}")
                for ci in range(CH):
                    nc.tensor.matmul(
                        ps, xtbf[ci][:, nj * P:(nj + 1) * P], wbf["v"][ci],
                        start=(ci == 0), stop=(ci == CH - 1),
                    )
                if nj % 2 == 0:
                    nc.scalar.activation(out=vbf[nj], in_=ps, func=AF.Copy)
                else:
                    nc.vector.tensor_copy(out=vbf[nj], in_=ps)

            # ---------- attention + projection, per 512-query block ----------
            for ib in range(QS):
                isl = slice(ib * FT, (ib + 1) * FT)
                # S^T tiles -> exp -> P~^T (bf16)
                pt = [
                    st.tile([P, FT], bf, tag=f"pt{j}", name=f"pt{ib}_{j}", bufs=1)
                    for j in range(NJ)
                ]
                for j in range(NJ):
                    ps = psb.tile([P, FT], fp, tag="big", name=f"ps_s{ib}_{j}")
                    for c in range(CH):
                        nc.tensor.matmul(
                            ps, ktbf[c][:, j * P:(j + 1) * P], qtbf[c][:, isl],
                            start=(c == 0), stop=(c == CH - 1),
                        )
                    nc.scalar.activation(out=pt[j], in_=ps, func=AF.Exp)
                # softmax denominator: ones^T @ P~^T, then reciprocal+broadcast
                ps_d = pss.tile([1, FT], fp, tag="denom", name=f"ps_d{ib}")
                for j in range(NJ):
                    nc.tensor.matmul(ps_d, ones_b[:, 0:1], pt[j], start=(j == 0), stop=(j == NJ - 1))
                rd_row = st.tile([1, FT], fp, tag="rd_row", name=f"rd_row{ib}")
                nc.vector.reciprocal(out=rd_row, in_=ps_d)
                ps_bc = psb.tile([P, FT], fp, tag="big", name=f"ps_rbc{ib}")
                nc.tensor.matmul(ps_bc, ones_f[0:1, :], rd_row, start=True, stop=True)
                rd_bc = st.tile([P, FT], fp, tag="rd_bc", name=f"rd_bc{ib}")
                nc.vector.tensor_copy(out=rd_bc, in_=ps_bc)
                # O^T[c, i] = sum_j V[j,c-chunk]^T P~^T[j, i], then /denom
                otbf = []
                for c in range(CH):
                    ps = psb.tile([P, FT], fp, tag="big", name=f"ps_o{ib}_{c}")
                    for j in range(NJ):
                        nc.tensor.matmul(
                            ps, vbf[j][:, c * P:(c + 1) * P], pt[j],
                            start=(j == 0), stop=(j == NJ - 1),
                        )
                    ot = st.tile([P, FT], bf, tag=f"ot{c}", name=f"ot{ib}_{c}", bufs=1)
                    nc.vector.tensor_mul(ot, ps, rd_bc)
                    otbf.append(ot)
                # Y^T[co, i] = wp^T-chunks @ O^T + bias' + residual
                for co in range(CH):
                    ps = psb.tile([P, FT], fp, tag="big", name=f"ps_y{ib}_{co}")
                    for c in range(CH):
                        nc.tensor.matmul(
                            ps, wbf["p"][c][:, co * P:(co + 1) * P], otbf[c],
                            start=(c == 0), stop=(c == CH - 1),
                        )
                    res = st.tile([P, FT], fp, tag="res", name=f"res{ib}_{co}", bufs=1)
                    nc.sync.dma_start(out=res, in_=xq_ext[co * P:(co + 1) * P, isl])
                    yt = st.tile([P, FT], fp, tag="yt", name=f"yt{ib}_{co}")
                    nc.vector.tensor_scalar(
                        out=yt, in0=ps, scalar1=biasp[co], scalar2=None, op0=ALU.add
                    )
                    nc.vector.tensor_add(yt, yt, res)
                    nc.sync.dma_start(out=out_ext[co * P:(co + 1) * P, isl], in_=yt)

    nc.finalize()
    return nc


def _get_nc():
    if "nc" not in _CACHE:
        _CACHE["nc"] = _build()
    return _CACHE["nc"]


def kernel(x, gamma, beta, wq, bq, wk, bk, wv, bv, wp, bp):
    from concourse.bass_utils import run_bass_kernel_spmd

    nc = _get_nc()

    x = np.asarray(x, dtype=np.float32)
    fmat = np.zeros((C, G), np.float32)
    emat = np.zeros((G, C), np.float32)
    for c in range(C):
        fmat[c, c // CPG] = 1.0 / CPG
        emat[c // CPG, c] = 1.0
    ones = np.ones((P, P), np.float32)

    def colv(v):
        return np.ascontiguousarray(np.asarray(v, np.float32).reshape(C, 1))

    vecs = np.concatenate(
        [colv(gamma), colv(beta), colv(bq), colv(bk), colv(bv), colv(bp)], axis=1
    )
    common = {
        "wq": np.asarray(wq, np.float32), "wk": np.asarray(wk, np.float32),
        "wv": np.asarray(wv, np.float32), "wp": np.asarray(wp, np.float32),
        "vecs": np.ascontiguousarray(vecs),
        "fmat": fmat, "emat": emat, "ones": ones,
    }

    xT = [np.ascontiguousarray(x[b].reshape(N, C).T) for b in range(B)]
    in_maps = []
    for core in range(8):
        b, r = core // 4, core % 4
        m = dict(common)
        m["xT"] = xT[b]
        m["xq"] = np.ascontiguousarray(xT[b][:, r * NQ:(r + 1) * NQ])
        in_maps.append(m)

    res = run_bass_kernel_spmd(nc, in_maps, core_ids=list(range(8)))

    out = np.empty((B, N, C), np.float32)
    for core in range(8):
        b, r = core // 4, core % 4
        out[b, r * NQ:(r + 1) * NQ, :] = res.results[core]["out"].T
    return out.reshape(B, Hh, Ww, C)
